# revision 1
# baseline (speedup 1.0000x reference)
"""Masked multi-head attention + residual + LayerNorm on 8 Trainium2 cores.

Sharding: phase 1 is (batch x head-group)-parallel: core c handles batch
c//2 and head-group c%2 (8 of 16 heads), computing q/k/v projections plus
causal softmax attention for one batch, emitting the attention output
TRANSPOSED ([head_dim, tokens]) so it feeds the output projection as lhsT
with no on-chip transposes anywhere. Phase 2 is token-parallel (1024 of
8192 token rows per core): output projection, bias, residual, LayerNorm.

Layout tricks:
  - Host pre-transposes x to x^T [D, S] per batch (needed as the moving
    operand of the QK projections and as lhsT of the V projection).
  - Scores are computed transposed ([keys, queries]); softmax exp runs on
    the scalar engine; the denominator comes from an extra all-ones column
    appended to v, so attn @ v and the row sums come out of one PSUM
    accumulation group.
  - Softmax skips max-subtraction (scores are O(1) here by construction;
    exp is safely in range), matching the reference up to fp rounding.
"""

import numpy as np

import concourse.bass as bass
import concourse.bacc as bacc
import concourse.mybir as mybir
from concourse.tile import TileContext

F32 = mybir.dt.float32
BF16 = mybir.dt.bfloat16
F32R = mybir.dt.float32r
B, S, D, H = 4, 2048, 1024, 16
HD = D // H          # 64
NC = 8               # cores
GW = D // 2          # 512: per-core head-group width (8 heads)
HPC = 8              # heads per core
T = B * S            # 8192 tokens
TPC = T // NC        # 1024 tokens per core (phase 2)
EPS = 1e-5
NEG = -1e30
QC = 512             # query chunk (psum free width)
KT = 128             # key tile (psum partition width)
NKD = D // 128       # 8 contraction tiles over model dim
NDT = GW // 128      # 4 projection-dim tiles per core
NTT = S // 128       # 16 token tiles per batch
NQB = S // QC        # 4 query chunks per batch


def _build_phase1(rep: int = 1) -> bass.Bass:
    nc = bacc.Bacc(None)
    xt = nc.dram_tensor("xt", [D, S], F32R, kind="ExternalInput")
    wq = nc.dram_tensor("wq", [D, GW], F32R, kind="ExternalInput")
    wk = nc.dram_tensor("wk", [D, GW], F32R, kind="ExternalInput")
    wv = nc.dram_tensor("wv", [D, GW], F32R, kind="ExternalInput")
    bq = nc.dram_tensor("bq", [GW], F32, kind="ExternalInput")  # pre-scaled 1/8
    bk = nc.dram_tensor("bk", [GW], F32, kind="ExternalInput")
    bv = nc.dram_tensor("bv", [GW], F32, kind="ExternalInput")
    masks = nc.dram_tensor("masks", [KT, 3 * KT + QC], F32, kind="ExternalInput")
    attn_t = nc.dram_tensor("attn_t", [GW, S], F32R, kind="ExternalOutput")

    with TileContext(nc) as tc:
        with (
            tc.tile_pool(name="const", bufs=1) as const,
            tc.tile_pool(name="xtp", bufs=1) as xtp,
            tc.tile_pool(name="qk", bufs=1) as qkp,
            tc.tile_pool(name="vp", bufs=1) as vp,
            tc.tile_pool(name="wstream", bufs=16) as wsp,
            tc.tile_pool(name="pt", bufs=8) as ptp,
            tc.tile_pool(name="small", bufs=6) as smallp,
            tc.tile_pool(name="ao", bufs=4) as aop,
            tc.tile_pool(name="mm", bufs=2, space="PSUM") as mmp,
            tc.tile_pool(name="sc", bufs=4, space="PSUM") as scp,
            tc.tile_pool(name="acc", bufs=2, space="PSUM") as accp,
        ):
            # --- constants ---
            bq_sb = const.tile([128, NDT], F32)
            nc.sync.dma_start(out=bq_sb, in_=bq.rearrange("(t p) -> p t", p=128))
            bk_sb = const.tile([128, NDT], F32)
            nc.sync.dma_start(out=bk_sb, in_=bk.rearrange("(t p) -> p t", p=128))
            bv_bc = const.tile([128, GW], F32)
            bv_ap = bv[:]
            nc.gpsimd.dma_start(
                out=bv_bc,
                in_=bass.AP(tensor=bv_ap.tensor, offset=bv_ap.offset,
                            ap=[[0, 128]] + bv_ap.ap))
            mask_sb = const.tile([KT, 3 * KT + QC], F32)
            nc.sync.dma_start(out=mask_sb, in_=masks[:, :])
            ones_sb = const.tile([1, HD], F32R)
            nc.vector.memset(ones_sb.bitcast(F32), 1.0)
            wv_sb = const.tile([128, NKD, GW], F32R)
            nc.sync.dma_start(out=wv_sb, in_=wv.rearrange("(k p) m -> p k m", p=128))

            def body():
                _p1_body(nc, tc, xtp, qkp, vp, wsp, ptp, smallp, aop,
                         mmp, scp, accp,
                         xt, wq, wk, wv_sb, attn_t,
                         bq_sb, bk_sb, bv_bc, mask_sb, ones_sb)

            if rep > 1:
                with tc.For_i(0, rep, 1):
                    body()
            else:
                body()
    nc.finalize()
    return nc


def _p1_body(nc, tc, xtp, qkp, vp, wsp, ptp, smallp, aop,
             mmp, scp, accp,
             xt, wq, wk, wv_sb, attn_t,
             bq_sb, bk_sb, bv_bc, mask_sb, ones_sb):
    if True:
        if True:
            # x^T, 8 partition tiles of [128, S]; all first halves land
            # before any second half so early proj chunks unblock sooner
            xts = []
            for kd in range(NKD):
                xt_sb = xtp.tile([128, S], F32R, tag=f"xt{kd}", name=f"xt{kd}")
                nc.sync.dma_start(out=xt_sb[:, 0:S // 2],
                                  in_=xt[kd * 128:(kd + 1) * 128, 0:S // 2])
                xts.append(xt_sb)
            for kd in range(NKD):
                nc.sync.dma_start(out=xts[kd][:, S // 2:S],
                                  in_=xt[kd * 128:(kd + 1) * 128, S // 2:S])

            # q^T, k^T : per dim-tile [128, S]; head h lives at partitions
            # (h%2)*64.., dim-tile h//2
            qts, kts = [], []
            for dt in range(NDT):
                qts.append(qkp.tile([128, S], BF16, tag=f"qt{dt}", name=f"qt{dt}"))
                kts.append(qkp.tile([128, S], BF16, tag=f"kt{dt}", name=f"kt{dt}"))

            def proj_qk_chunks(dt):
                """Yield once per (w, nch) chunk so callers can interleave."""
                dsl = slice(dt * 128, (dt + 1) * 128)
                for w_dram, dst, bias in ((wq, qts[dt], bq_sb), (wk, kts[dt], bk_sb)):
                    wtiles = []
                    for kd in range(NKD):
                        wt = wsp.tile([128, 128], F32R, tag="w", name="wt")
                        nc.sync.dma_start(
                            out=wt, in_=w_dram[kd * 128:(kd + 1) * 128, dsl])
                        wtiles.append(wt)
                    for nch in range(NQB):
                        sl = slice(nch * QC, (nch + 1) * QC)
                        pq = mmp.tile([128, QC], F32, tag="mm", name="pq")
                        for kd in range(NKD):
                            nc.tensor.matmul(pq, lhsT=wtiles[kd],
                                             rhs=xts[kd][:, sl],
                                             start=(kd == 0), stop=(kd == NKD - 1))
                        nc.vector.tensor_scalar(
                            out=dst[:, sl], in0=pq,
                            scalar1=bias[:, dt:dt + 1], scalar2=None,
                            op0=mybir.AluOpType.add)
                        yield

            def proj_qk(dt):
                for _ in proj_qk_chunks(dt):
                    pass

            # v natural [tokens, dims] + per-head all-ones column:
            # per token-tile [128, 8*65]; head h slice = [:, h*65:h*65+65]
            v_tiles = []

            def proj_v():
                for tt in range(NTT):
                    tsl = slice(tt * 128, (tt + 1) * 128)
                    v_sb = vp.tile([128, HPC * (HD + 1)], F32R,
                                   tag=f"v{tt}", name=f"v{tt}")
                    v_tiles.append(v_sb)
                    ocols = bass.AP(tensor=v_sb.tensor, offset=v_sb.offset + HD,
                                    ap=[v_sb.ap[0], [(HD + 1), HPC], [1, 1]])
                    nc.vector.memset(ocols.bitcast(F32), 1.0)
                    pv = mmp.tile([128, GW], F32, tag="mm", name="pv")
                    for kd in range(NKD):
                        nc.tensor.matmul(pv, lhsT=xts[kd][:, tsl],
                                         rhs=wv_sb[:, kd, :],
                                         start=(kd == 0), stop=(kd == NKD - 1))
                    for h in range(HPC):
                        nc.vector.tensor_add(
                            v_sb[:, h * (HD + 1):h * (HD + 1) + HD],
                            pv[:, h * HD:(h + 1) * HD],
                            bv_bc[:, h * HD:(h + 1) * HD])

            def attention(h, filler=None):
                po = (h % 2) * HD          # partition offset within dim-tile
                dt = h // 2
                vofs = h * (HD + 1)
                for qb in range(NQB):
                    if filler is not None:
                        next(filler, None)
                    qsl = slice(qb * QC, (qb + 1) * QC)
                    nkt = (qb + 1) * (QC // KT)
                    acc = accp.tile([HD + 1, QC], F32, tag="acc", name="acc")
                    for ktile in range(nkt):
                        ksl = slice(ktile * KT, (ktile + 1) * KT)
                        r = ktile - qb * (QC // KT)
                        # queries below 128*r in this chunk are fully masked
                        qo = max(r, 0) * KT
                        qslr = slice(qb * QC + qo, (qb + 1) * QC)
                        ps = scp.tile([KT, QC], F32, tag="sc", name="ps")
                        nc.tensor.matmul(ps[:, qo:QC],
                                         lhsT=kts[dt][po:po + HD, ksl],
                                         rhs=qts[dt][po:po + HD, qslr],
                                         start=True, stop=True)
                        pt = ptp.tile([KT, QC], F32R, tag="pt", name="pt")
                        if r >= 0:  # boundary block: causal mask
                            mo = 3 * KT - r * KT
                            nc.vector.tensor_add(pt[:, qo:QC], ps[:, qo:QC],
                                                 mask_sb[:, mo + qo:mo + QC])
                            nc.scalar.activation(
                                pt[:, qo:QC], pt[:, qo:QC],
                                mybir.ActivationFunctionType.Exp)
                        else:
                            nc.scalar.activation(
                                pt, ps, mybir.ActivationFunctionType.Exp)
                        nc.tensor.matmul(
                            acc[:, qo:QC],
                            lhsT=v_tiles[ktile][:, vofs:vofs + HD + 1],
                            rhs=pt[:, qo:QC],
                            start=(ktile == 0), stop=(ktile == nkt - 1))
                    rsr = smallp.tile([1, QC], F32R, tag="rsr", name="rsr")
                    with nc.allow_low_precision(reason="f32r recip, 1e-4 ok"):
                        nc.vector.reciprocal(rsr, acc[HD:HD + 1, :])
                    bc = mmp.tile([HD, QC], F32, tag="mm", name="bc")
                    nc.tensor.matmul(bc, lhsT=ones_sb, rhs=rsr, start=True, stop=True)
                    ao = aop.tile([HD, QC], F32R, tag="ao", name="ao")
                    nc.vector.tensor_copy(ao, acc[0:HD, :])
                    nc.vector.tensor_mul(ao, ao, bc)
                    nc.sync.dma_start(
                        out=attn_t[h * HD:(h + 1) * HD, qsl], in_=ao)

            # interleave: during attention of heads 2dt/2dt+1, sprinkle the
            # next dim-tile's projection chunks to keep PE dense
            proj_qk(0)
            proj_v()
            for dt in range(NDT):
                filler = proj_qk_chunks(dt + 1) if dt + 1 < NDT else iter(())
                attention(2 * dt, filler)
                attention(2 * dt + 1, filler)


def _build_phase2(rep: int = 1) -> bass.Bass:
    nc = bacc.Bacc(None)
    at = nc.dram_tensor("at", [D, TPC], F32R, kind="ExternalInput")   # attn^T slice
    wo = nc.dram_tensor("wo", [D, D], F32R, kind="ExternalInput")
    xr = nc.dram_tensor("xr", [TPC, D], F32, kind="ExternalInput")   # residual rows
    bo = nc.dram_tensor("bo", [D], F32, kind="ExternalInput")
    lng = nc.dram_tensor("lng", [D], F32, kind="ExternalInput")
    lnb = nc.dram_tensor("lnb", [D], F32, kind="ExternalInput")
    out = nc.dram_tensor("out", [TPC, D], F32, kind="ExternalOutput")

    NMT = TPC // 128    # 8 token tiles
    NNC = D // QC       # 2 output column chunks

    with TileContext(nc) as tc:
        with (
            tc.tile_pool(name="const", bufs=1) as const,
            tc.tile_pool(name="work", bufs=3) as work,
            tc.tile_pool(name="stat", bufs=4) as statp,
            tc.tile_pool(name="pp", bufs=2, space="PSUM") as pp,
        ):
            at_sb = const.tile([128, NKD, TPC], F32R)
            nc.sync.dma_start(out=at_sb, in_=at.rearrange("(k p) m -> p k m", p=128))
            wo_sb = const.tile([128, NKD, D], F32R)
            nc.sync.dma_start(out=wo_sb, in_=wo.rearrange("(k p) n -> p k n", p=128))
            x_sb = const.tile([128, NMT, D], F32)
            nc.sync.dma_start(out=x_sb, in_=xr.rearrange("(m p) d -> p m d", p=128))

            def bcast(v):
                a = v[:]
                t = const.tile([128, D], F32, name=f"{v.name}_bc")
                nc.gpsimd.dma_start(
                    out=t,
                    in_=bass.AP(tensor=a.tensor, offset=a.offset,
                                ap=[[0, 128]] + a.ap))
                return t

            bo_bc, lng_bc, lnb_bc = bcast(bo), bcast(lng), bcast(lnb)
            eps_sb = const.tile([128, 1], F32)
            nc.vector.memset(eps_sb, EPS)

            def body():
                _p2_body(nc, work, statp, pp, at_sb, wo_sb, x_sb,
                         bo_bc, lng_bc, lnb_bc, eps_sb, out)

            if rep > 1:
                with tc.For_i(0, rep, 1):
                    body()
            else:
                body()
    nc.finalize()
    return nc


def _p2_body(nc, work, statp, pp, at_sb, wo_sb, x_sb,
             bo_bc, lng_bc, lnb_bc, eps_sb, out):
    NMT = TPC // 128
    NNC = D // QC
    if True:
        if True:
            for mt in range(NMT):
                msl = slice(mt * 128, (mt + 1) * 128)
                res = work.tile([128, D], F32, tag="res", name="res")
                for nchunk in range(NNC):
                    nsl = slice(nchunk * QC, (nchunk + 1) * QC)
                    ps = pp.tile([128, QC], F32, tag="pp", name="ps")
                    for kd in range(NKD):
                        nc.tensor.matmul(ps, lhsT=at_sb[:, kd, msl],
                                         rhs=wo_sb[:, kd, nsl],
                                         start=(kd == 0), stop=(kd == NKD - 1))
                    nc.vector.tensor_add(res[:, nsl], ps, bo_bc[:, nsl])
                nc.vector.tensor_add(res, res, x_sb[:, mt, :])
                # layernorm over free dim (D=1024 -> 2 bn_stats subgroups)
                stats = statp.tile([128, 2, 6], F32, tag="stats", name="stats")
                nc.vector.bn_stats(out=stats[:, 0, :], in_=res[:, 0:512])
                nc.vector.bn_stats(out=stats[:, 1, :], in_=res[:, 512:1024])
                mv = statp.tile([128, 2], F32, tag="mv", name="mv")
                nc.vector.bn_aggr(out=mv, in_=stats)
                rstd = statp.tile([128, 1], F32, tag="rstd", name="rstd")
                nc.scalar.activation(rstd, mv[:, 1:2],
                                     mybir.ActivationFunctionType.Sqrt,
                                     bias=eps_sb, scale=1.0)
                nc.vector.reciprocal(rstd, rstd)
                nc.vector.tensor_scalar(
                    out=res, in0=res, scalar1=mv[:, 0:1], scalar2=rstd,
                    op0=mybir.AluOpType.subtract, op1=mybir.AluOpType.mult)
                nc.vector.tensor_mul(res, res, lng_bc)
                nc.vector.tensor_add(res, res, lnb_bc)
                nc.sync.dma_start(out=out[msl, :], in_=res)


_CACHE = {}


class _Runner:
    """Reusable jitted SPMD runner for a finalized Bass program.

    Mirrors concourse.bass2jax.run_bass_via_pjrt's multi-core path, but
    caches the jitted callable so repeat kernel() calls skip re-tracing
    and NEFF reload. Also exposes a device-resident benchmark mode.
    """

    def __init__(self, nc):
        import jax
        from jax.experimental.shard_map import shard_map
        from jax.sharding import Mesh, PartitionSpec
        from concourse import mybir as _mybir
        from concourse import bass2jax as _b2j

        _b2j.install_neuronx_cc_hook()
        self.jax = jax

        in_names, out_names, out_avals = [], [], []
        partition_name = (nc.partition_id_tensor.name
                          if nc.partition_id_tensor else None)
        for alloc in nc.m.functions[0].allocations:
            if not isinstance(alloc, _mybir.MemoryLocationSet):
                continue
            name = alloc.memorylocations[0].name
            if alloc.kind == "ExternalInput":
                if name != partition_name:
                    in_names.append(name)
            elif alloc.kind == "ExternalOutput":
                out_avals.append(
                    jax.core.ShapedArray(tuple(alloc.tensor_shape),
                                         _mybir.dt.np(alloc.dtype)))
                out_names.append(name)
        n_params = len(in_names)
        n_outs = len(out_avals)
        all_in_names = list(in_names) + list(out_names)
        if partition_name is not None:
            all_in_names.append(partition_name)
        donate = tuple(range(n_params, n_params + n_outs))

        def _body(*args):
            operands = list(args)
            if partition_name is not None:
                operands.append(_b2j.partition_id_tensor())
            outs = _b2j._bass_exec_p.bind(
                *operands,
                out_avals=tuple(out_avals),
                in_names=tuple(all_in_names),
                out_names=tuple(out_names),
                lowering_input_output_aliases=(),
                sim_require_finite=True,
                sim_require_nnan=True,
                nc=nc,
            )
            return tuple(outs)

        devices = jax.devices()[:NC]
        self.mesh = Mesh(np.asarray(devices), ("core",))
        self.pspec = PartitionSpec("core")
        in_specs = (self.pspec,) * (n_params + n_outs)
        out_specs = (self.pspec,) * n_outs
        self.sharded = jax.jit(
            shard_map(_body, mesh=self.mesh, in_specs=in_specs,
                      out_specs=out_specs, check_rep=False),
            donate_argnums=donate, keep_unused=True)
        self.in_names = in_names
        self.out_names = out_names
        self.out_avals = out_avals

    def _concat_in(self, in_maps):
        return [
            np.concatenate([np.asarray(m[name]) for m in in_maps], axis=0)
            for name in self.in_names
        ]

    def _zeros(self):
        return [np.zeros((NC * a.shape[0], *a.shape[1:]), a.dtype)
                for a in self.out_avals]

    def run(self, in_maps):
        out_arrs = self.sharded(*self._concat_in(in_maps), *self._zeros())
        self.jax.block_until_ready(out_arrs)
        return [
            {name: np.asarray(out_arrs[i]).reshape(NC, *self.out_avals[i].shape)[c]
             for i, name in enumerate(self.out_names)}
            for c in range(NC)
        ]

    def bench(self, in_maps, iters=5):
        """Time steady-state execution with device-resident inputs."""
        import time
        jax = self.jax
        from jax.sharding import NamedSharding
        sh = NamedSharding(self.mesh, self.pspec)
        dev_in = [jax.device_put(a, sh) for a in self._concat_in(in_maps)]
        jax.block_until_ready(dev_in)
        times = []
        for _ in range(iters):
            zs = [jax.device_put(z, sh) for z in self._zeros()]
            jax.block_until_ready(zs)
            t0 = time.perf_counter()
            out = self.sharded(*dev_in, *zs)
            jax.block_until_ready(out)
            times.append(time.perf_counter() - t0)
        return min(times), times


def _programs():
    if "run1" not in _CACHE:
        _CACHE["run1"] = _Runner(_build_phase1())
        _CACHE["run2"] = _Runner(_build_phase2())
    return _CACHE["run1"], _CACHE["run2"]


def _masks() -> np.ndarray:
    # sliding-window causal mask: variant r = W[:, 3*KT - r*KT :][:QC]
    # W[j, u] = 0 if j <= u - 3*KT else NEG
    W = np.zeros((KT, 3 * KT + QC), dtype=np.float32)
    j = np.arange(KT)[:, None]
    u = np.arange(3 * KT + QC)[None, :]
    W[j > u - 3 * KT] = NEG
    return W


def _phase1_inputs(x, wq, bq, wk, bk, wv, bv):
    xt = np.ascontiguousarray(x.transpose(0, 2, 1))        # [B, D, S]
    masks = _masks()
    in1 = []
    for c in range(NC):
        b, g = c // 2, c % 2
        sl = slice(g * GW, (g + 1) * GW)
        in1.append({
            "xt": xt[b],
            "wq": np.ascontiguousarray(np.asarray(wq)[:, sl]) * np.float32(0.125),
            "wk": np.ascontiguousarray(np.asarray(wk)[:, sl]),
            "wv": np.ascontiguousarray(np.asarray(wv)[:, sl]),
            "bq": np.ascontiguousarray(np.asarray(bq)[sl]) * np.float32(0.125),
            "bk": np.ascontiguousarray(np.asarray(bk)[sl]),
            "bv": np.ascontiguousarray(np.asarray(bv)[sl]),
            "masks": masks,
        })
    return in1


def _phase2_inputs(x, attn_full, wo, bo, ln_g, ln_b):
    x_flat = x.reshape(T, D)
    in2 = []
    for c in range(NC):
        sl = slice(c * TPC, (c + 1) * TPC)
        in2.append({
            "at": np.ascontiguousarray(attn_full[:, sl]),
            "wo": np.asarray(wo, np.float32),
            "xr": x_flat[sl],
            "bo": np.asarray(bo, np.float32),
            "lng": np.asarray(ln_g, np.float32),
            "lnb": np.asarray(ln_b, np.float32),
        })
    return in2


def _assemble_attn(r1):
    attn_full = np.empty((D, T), np.float32)
    for c in range(NC):
        b, g = c // 2, c % 2
        attn_full[g * GW:(g + 1) * GW, b * S:(b + 1) * S] = r1[c]["attn_t"]
    return attn_full


def kernel(x, wq, bq, wk, bk, wv, bv, wo, bo, ln_g, ln_b, _profile=None):
    import time as _time
    x = np.asarray(x, np.float32)
    run1, run2 = _programs()

    in1 = _phase1_inputs(x, wq, bq, wk, bk, wv, bv)
    t0 = _time.perf_counter()
    r1 = run1.run(in1)
    t1 = _time.perf_counter()
    attn_full = _assemble_attn(r1)
    in2 = _phase2_inputs(x, attn_full, wo, bo, ln_g, ln_b)
    t2 = _time.perf_counter()
    r2 = run2.run(in2)
    t3 = _time.perf_counter()
    out = np.concatenate([r2[c]["out"] for c in range(NC)], axis=0)
    if _profile is not None:
        _profile["t_phase1"] = t1 - t0
        _profile["t_phase2"] = t3 - t2
        _profile["in1"], _profile["in2"] = in1, in2
    return out.reshape(B, S, D)



# revision 5
# speedup vs baseline: 2.2197x; 2.2197x over previous
"""Masked multi-head attention + residual + LayerNorm on 8 Trainium2 cores.

Sharding: phase 1 is (batch x head-group)-parallel: core c handles batch
c//2 and head-group c%2 (8 of 16 heads), computing q/k/v projections plus
causal softmax attention for one batch, emitting the attention output
TRANSPOSED ([head_dim, tokens]) so it feeds the output projection as lhsT
with no on-chip transposes anywhere. Phase 2 is token-parallel (1024 of
8192 token rows per core): output projection, bias, residual, LayerNorm.

Dispatch: the whole thing is ONE asynchronously-dispatched chain —
phase-1 NEFF -> on-device reshard (XLA all-to-all between the two cores
of each batch) -> phase-2 NEFF — with a single block at the end. The
axon tunnel has ~83 ms of round-trip latency per synchronization, so the
chain pays it once instead of twice (the old host-side attn exchange
forced a second round trip). Output zero-buffers (donated to the NEFFs)
are created device-side by tiny jitted programs rather than uploaded.

Layout tricks:
  - Host pre-transposes x to x^T [D, S] per batch (needed as the moving
    operand of the QK projections and as lhsT of the V projection).
  - Scores are computed transposed ([keys, queries]); softmax exp runs on
    the scalar engine; the denominator comes from an extra all-ones column
    appended to v, so attn @ v and the row sums come out of one PSUM
    accumulation group.
  - Softmax skips max-subtraction (scores are O(1) here by construction;
    exp is safely in range), matching the reference up to fp rounding.
"""

import numpy as np

import concourse.bass as bass
import concourse.bacc as bacc
import concourse.mybir as mybir
from concourse.tile import TileContext

F32 = mybir.dt.float32
BF16 = mybir.dt.bfloat16
F32R = mybir.dt.float32r
B, S, D, H = 4, 2048, 1024, 16
HD = D // H          # 64
NC = 8               # cores
GW = D // 2          # 512: per-core head-group width (8 heads)
HPC = 8              # heads per core
T = B * S            # 8192 tokens
TPC = T // NC        # 1024 tokens per core (phase 2)
EPS = 1e-5
NEG = -1e30
QC = 512             # query chunk (psum free width)
KT = 128             # key tile (psum partition width)
NKD = D // 128       # 8 contraction tiles over model dim
NDT = GW // 128      # 4 projection-dim tiles per core
NTT = S // 128       # 16 token tiles per batch
NQB = S // QC        # 4 query chunks per batch


def _build_phase1(rep: int = 1) -> bass.Bass:
    nc = bacc.Bacc(None)
    xt = nc.dram_tensor("xt", [D, S], F32R, kind="ExternalInput")
    wq = nc.dram_tensor("wq", [D, GW], F32R, kind="ExternalInput")
    wk = nc.dram_tensor("wk", [D, GW], F32R, kind="ExternalInput")
    wv = nc.dram_tensor("wv", [D, GW], F32R, kind="ExternalInput")
    bq = nc.dram_tensor("bq", [GW], F32, kind="ExternalInput")  # pre-scaled 1/8
    bk = nc.dram_tensor("bk", [GW], F32, kind="ExternalInput")
    bv = nc.dram_tensor("bv", [GW], F32, kind="ExternalInput")
    masks = nc.dram_tensor("masks", [KT, 3 * KT + QC], F32, kind="ExternalInput")
    attn_t = nc.dram_tensor("attn_t", [GW, S], F32R, kind="ExternalOutput")

    with TileContext(nc) as tc:
        with (
            tc.tile_pool(name="const", bufs=1) as const,
            tc.tile_pool(name="xtp", bufs=1) as xtp,
            tc.tile_pool(name="qk", bufs=1) as qkp,
            tc.tile_pool(name="vp", bufs=1) as vp,
            tc.tile_pool(name="wstream", bufs=16) as wsp,
            tc.tile_pool(name="pt", bufs=8) as ptp,
            tc.tile_pool(name="small", bufs=6) as smallp,
            tc.tile_pool(name="ao", bufs=4) as aop,
            tc.tile_pool(name="mm", bufs=2, space="PSUM") as mmp,
            tc.tile_pool(name="sc", bufs=4, space="PSUM") as scp,
            tc.tile_pool(name="acc", bufs=2, space="PSUM") as accp,
        ):
            # --- constants ---
            bq_sb = const.tile([128, NDT], F32)
            nc.sync.dma_start(out=bq_sb, in_=bq.rearrange("(t p) -> p t", p=128))
            bk_sb = const.tile([128, NDT], F32)
            nc.sync.dma_start(out=bk_sb, in_=bk.rearrange("(t p) -> p t", p=128))
            bv_bc = const.tile([128, GW], F32)
            bv_ap = bv[:]
            nc.gpsimd.dma_start(
                out=bv_bc,
                in_=bass.AP(tensor=bv_ap.tensor, offset=bv_ap.offset,
                            ap=[[0, 128]] + bv_ap.ap))
            mask_sb = const.tile([KT, 3 * KT + QC], F32)
            nc.sync.dma_start(out=mask_sb, in_=masks[:, :])
            ones_sb = const.tile([1, HD], F32R)
            nc.vector.memset(ones_sb.bitcast(F32), 1.0)
            wv_sb = const.tile([128, NKD, GW], F32R)
            nc.sync.dma_start(out=wv_sb, in_=wv.rearrange("(k p) m -> p k m", p=128))

            def body():
                _p1_body(nc, tc, xtp, qkp, vp, wsp, ptp, smallp, aop,
                         mmp, scp, accp,
                         xt, wq, wk, wv_sb, attn_t,
                         bq_sb, bk_sb, bv_bc, mask_sb, ones_sb)

            if rep > 1:
                with tc.For_i(0, rep, 1):
                    body()
            else:
                body()
    nc.finalize()
    return nc


def _p1_body(nc, tc, xtp, qkp, vp, wsp, ptp, smallp, aop,
             mmp, scp, accp,
             xt, wq, wk, wv_sb, attn_t,
             bq_sb, bk_sb, bv_bc, mask_sb, ones_sb):
    if True:
        if True:
            # x^T, 8 partition tiles of [128, S]; all first halves land
            # before any second half so early proj chunks unblock sooner
            xts = []
            for kd in range(NKD):
                xt_sb = xtp.tile([128, S], F32R, tag=f"xt{kd}", name=f"xt{kd}")
                nc.sync.dma_start(out=xt_sb[:, 0:S // 2],
                                  in_=xt[kd * 128:(kd + 1) * 128, 0:S // 2])
                xts.append(xt_sb)
            for kd in range(NKD):
                nc.sync.dma_start(out=xts[kd][:, S // 2:S],
                                  in_=xt[kd * 128:(kd + 1) * 128, S // 2:S])

            # q^T, k^T : per dim-tile [128, S]; head h lives at partitions
            # (h%2)*64.., dim-tile h//2
            qts, kts = [], []
            for dt in range(NDT):
                qts.append(qkp.tile([128, S], BF16, tag=f"qt{dt}", name=f"qt{dt}"))
                kts.append(qkp.tile([128, S], BF16, tag=f"kt{dt}", name=f"kt{dt}"))

            def proj_qk_chunks(dt):
                """Yield once per (w, nch) chunk so callers can interleave."""
                dsl = slice(dt * 128, (dt + 1) * 128)
                for w_dram, dst, bias in ((wq, qts[dt], bq_sb), (wk, kts[dt], bk_sb)):
                    wtiles = []
                    for kd in range(NKD):
                        wt = wsp.tile([128, 128], F32R, tag="w", name="wt")
                        nc.sync.dma_start(
                            out=wt, in_=w_dram[kd * 128:(kd + 1) * 128, dsl])
                        wtiles.append(wt)
                    for nch in range(NQB):
                        sl = slice(nch * QC, (nch + 1) * QC)
                        pq = mmp.tile([128, QC], F32, tag="mm", name="pq")
                        for kd in range(NKD):
                            nc.tensor.matmul(pq, lhsT=wtiles[kd],
                                             rhs=xts[kd][:, sl],
                                             start=(kd == 0), stop=(kd == NKD - 1))
                        nc.vector.tensor_scalar(
                            out=dst[:, sl], in0=pq,
                            scalar1=bias[:, dt:dt + 1], scalar2=None,
                            op0=mybir.AluOpType.add)
                        yield

            def proj_qk(dt):
                for _ in proj_qk_chunks(dt):
                    pass

            # v natural [tokens, dims] + per-head all-ones column:
            # per token-tile [128, 8*65]; head h slice = [:, h*65:h*65+65]
            v_tiles = []

            def proj_v():
                for tt in range(NTT):
                    tsl = slice(tt * 128, (tt + 1) * 128)
                    v_sb = vp.tile([128, HPC * (HD + 1)], F32R,
                                   tag=f"v{tt}", name=f"v{tt}")
                    v_tiles.append(v_sb)
                    ocols = bass.AP(tensor=v_sb.tensor, offset=v_sb.offset + HD,
                                    ap=[v_sb.ap[0], [(HD + 1), HPC], [1, 1]])
                    nc.vector.memset(ocols.bitcast(F32), 1.0)
                    pv = mmp.tile([128, GW], F32, tag="mm", name="pv")
                    for kd in range(NKD):
                        nc.tensor.matmul(pv, lhsT=xts[kd][:, tsl],
                                         rhs=wv_sb[:, kd, :],
                                         start=(kd == 0), stop=(kd == NKD - 1))
                    for h in range(HPC):
                        nc.vector.tensor_add(
                            v_sb[:, h * (HD + 1):h * (HD + 1) + HD],
                            pv[:, h * HD:(h + 1) * HD],
                            bv_bc[:, h * HD:(h + 1) * HD])

            def attention(h, filler=None):
                po = (h % 2) * HD          # partition offset within dim-tile
                dt = h // 2
                vofs = h * (HD + 1)
                for qb in range(NQB):
                    if filler is not None:
                        next(filler, None)
                    qsl = slice(qb * QC, (qb + 1) * QC)
                    nkt = (qb + 1) * (QC // KT)
                    acc = accp.tile([HD + 1, QC], F32, tag="acc", name="acc")
                    for ktile in range(nkt):
                        ksl = slice(ktile * KT, (ktile + 1) * KT)
                        r = ktile - qb * (QC // KT)
                        # queries below 128*r in this chunk are fully masked
                        qo = max(r, 0) * KT
                        qslr = slice(qb * QC + qo, (qb + 1) * QC)
                        ps = scp.tile([KT, QC], F32, tag="sc", name="ps")
                        nc.tensor.matmul(ps[:, qo:QC],
                                         lhsT=kts[dt][po:po + HD, ksl],
                                         rhs=qts[dt][po:po + HD, qslr],
                                         start=True, stop=True)
                        pt = ptp.tile([KT, QC], F32R, tag="pt", name="pt")
                        if r >= 0:  # boundary block: causal mask
                            mo = 3 * KT - r * KT
                            nc.vector.tensor_add(pt[:, qo:QC], ps[:, qo:QC],
                                                 mask_sb[:, mo + qo:mo + QC])
                            nc.scalar.activation(
                                pt[:, qo:QC], pt[:, qo:QC],
                                mybir.ActivationFunctionType.Exp)
                        else:
                            nc.scalar.activation(
                                pt, ps, mybir.ActivationFunctionType.Exp)
                        nc.tensor.matmul(
                            acc[:, qo:QC],
                            lhsT=v_tiles[ktile][:, vofs:vofs + HD + 1],
                            rhs=pt[:, qo:QC],
                            start=(ktile == 0), stop=(ktile == nkt - 1))
                    rsr = smallp.tile([1, QC], F32R, tag="rsr", name="rsr")
                    with nc.allow_low_precision(reason="f32r recip, 1e-4 ok"):
                        nc.vector.reciprocal(rsr, acc[HD:HD + 1, :])
                    bc = mmp.tile([HD, QC], F32, tag="mm", name="bc")
                    nc.tensor.matmul(bc, lhsT=ones_sb, rhs=rsr, start=True, stop=True)
                    ao = aop.tile([HD, QC], F32R, tag="ao", name="ao")
                    nc.vector.tensor_copy(ao, acc[0:HD, :])
                    nc.vector.tensor_mul(ao, ao, bc)
                    nc.sync.dma_start(
                        out=attn_t[h * HD:(h + 1) * HD, qsl], in_=ao)

            # interleave: during attention of heads 2dt/2dt+1, sprinkle the
            # next dim-tile's projection chunks to keep PE dense
            proj_qk(0)
            proj_v()
            for dt in range(NDT):
                filler = proj_qk_chunks(dt + 1) if dt + 1 < NDT else iter(())
                attention(2 * dt, filler)
                attention(2 * dt + 1, filler)


def _build_phase2(rep: int = 1) -> bass.Bass:
    nc = bacc.Bacc(None)
    at = nc.dram_tensor("at", [D, TPC], F32R, kind="ExternalInput")   # attn^T slice
    wo = nc.dram_tensor("wo", [D, D], F32R, kind="ExternalInput")
    xr = nc.dram_tensor("xr", [TPC, D], F32, kind="ExternalInput")   # residual rows
    bo = nc.dram_tensor("bo", [D], F32, kind="ExternalInput")
    lng = nc.dram_tensor("lng", [D], F32, kind="ExternalInput")
    lnb = nc.dram_tensor("lnb", [D], F32, kind="ExternalInput")
    out = nc.dram_tensor("out", [TPC, D], F32, kind="ExternalOutput")

    NMT = TPC // 128    # 8 token tiles
    NNC = D // QC       # 2 output column chunks

    with TileContext(nc) as tc:
        with (
            tc.tile_pool(name="const", bufs=1) as const,
            tc.tile_pool(name="work", bufs=3) as work,
            tc.tile_pool(name="stat", bufs=4) as statp,
            tc.tile_pool(name="pp", bufs=2, space="PSUM") as pp,
        ):
            at_sb = const.tile([128, NKD, TPC], F32R)
            nc.sync.dma_start(out=at_sb, in_=at.rearrange("(k p) m -> p k m", p=128))
            wo_sb = const.tile([128, NKD, D], F32R)
            nc.sync.dma_start(out=wo_sb, in_=wo.rearrange("(k p) n -> p k n", p=128))
            x_sb = const.tile([128, NMT, D], F32)
            nc.sync.dma_start(out=x_sb, in_=xr.rearrange("(m p) d -> p m d", p=128))

            def bcast(v):
                a = v[:]
                t = const.tile([128, D], F32, name=f"{v.name}_bc")
                nc.gpsimd.dma_start(
                    out=t,
                    in_=bass.AP(tensor=a.tensor, offset=a.offset,
                                ap=[[0, 128]] + a.ap))
                return t

            bo_bc, lng_bc, lnb_bc = bcast(bo), bcast(lng), bcast(lnb)
            eps_sb = const.tile([128, 1], F32)
            nc.vector.memset(eps_sb, EPS)

            def body():
                _p2_body(nc, work, statp, pp, at_sb, wo_sb, x_sb,
                         bo_bc, lng_bc, lnb_bc, eps_sb, out)

            if rep > 1:
                with tc.For_i(0, rep, 1):
                    body()
            else:
                body()
    nc.finalize()
    return nc


def _p2_body(nc, work, statp, pp, at_sb, wo_sb, x_sb,
             bo_bc, lng_bc, lnb_bc, eps_sb, out):
    NMT = TPC // 128
    NNC = D // QC
    if True:
        if True:
            for mt in range(NMT):
                msl = slice(mt * 128, (mt + 1) * 128)
                res = work.tile([128, D], F32, tag="res", name="res")
                for nchunk in range(NNC):
                    nsl = slice(nchunk * QC, (nchunk + 1) * QC)
                    ps = pp.tile([128, QC], F32, tag="pp", name="ps")
                    for kd in range(NKD):
                        nc.tensor.matmul(ps, lhsT=at_sb[:, kd, msl],
                                         rhs=wo_sb[:, kd, nsl],
                                         start=(kd == 0), stop=(kd == NKD - 1))
                    nc.vector.tensor_add(res[:, nsl], ps, bo_bc[:, nsl])
                nc.vector.tensor_add(res, res, x_sb[:, mt, :])
                # layernorm over free dim (D=1024 -> 2 bn_stats subgroups)
                stats = statp.tile([128, 2, 6], F32, tag="stats", name="stats")
                nc.vector.bn_stats(out=stats[:, 0, :], in_=res[:, 0:512])
                nc.vector.bn_stats(out=stats[:, 1, :], in_=res[:, 512:1024])
                mv = statp.tile([128, 2], F32, tag="mv", name="mv")
                nc.vector.bn_aggr(out=mv, in_=stats)
                rstd = statp.tile([128, 1], F32, tag="rstd", name="rstd")
                nc.scalar.activation(rstd, mv[:, 1:2],
                                     mybir.ActivationFunctionType.Sqrt,
                                     bias=eps_sb, scale=1.0)
                nc.vector.reciprocal(rstd, rstd)
                nc.vector.tensor_scalar(
                    out=res, in0=res, scalar1=mv[:, 0:1], scalar2=rstd,
                    op0=mybir.AluOpType.subtract, op1=mybir.AluOpType.mult)
                nc.vector.tensor_mul(res, res, lng_bc)
                nc.vector.tensor_add(res, res, lnb_bc)
                nc.sync.dma_start(out=out[msl, :], in_=res)


_CACHE = {}


class _Runner:
    """Reusable jitted SPMD runner for a finalized Bass program.

    Mirrors concourse.bass2jax.run_bass_via_pjrt's multi-core path, but
    caches the jitted callable so repeat kernel() calls skip re-tracing
    and NEFF reload. Also exposes a device-resident benchmark mode.
    """

    def __init__(self, nc):
        import jax
        from jax.experimental.shard_map import shard_map
        from jax.sharding import Mesh, PartitionSpec
        from concourse import mybir as _mybir
        from concourse import bass2jax as _b2j

        _b2j.install_neuronx_cc_hook()
        self.jax = jax

        in_names, out_names, out_avals = [], [], []
        partition_name = (nc.partition_id_tensor.name
                          if nc.partition_id_tensor else None)
        for alloc in nc.m.functions[0].allocations:
            if not isinstance(alloc, _mybir.MemoryLocationSet):
                continue
            name = alloc.memorylocations[0].name
            if alloc.kind == "ExternalInput":
                if name != partition_name:
                    in_names.append(name)
            elif alloc.kind == "ExternalOutput":
                out_avals.append(
                    jax.core.ShapedArray(tuple(alloc.tensor_shape),
                                         _mybir.dt.np(alloc.dtype)))
                out_names.append(name)
        n_params = len(in_names)
        n_outs = len(out_avals)
        all_in_names = list(in_names) + list(out_names)
        if partition_name is not None:
            all_in_names.append(partition_name)
        donate = tuple(range(n_params, n_params + n_outs))

        def _body(*args):
            operands = list(args)
            if partition_name is not None:
                operands.append(_b2j.partition_id_tensor())
            outs = _b2j._bass_exec_p.bind(
                *operands,
                out_avals=tuple(out_avals),
                in_names=tuple(all_in_names),
                out_names=tuple(out_names),
                lowering_input_output_aliases=(),
                sim_require_finite=True,
                sim_require_nnan=True,
                nc=nc,
            )
            return tuple(outs)

        devices = jax.devices()[:NC]
        self.mesh = Mesh(np.asarray(devices), ("core",))
        self.pspec = PartitionSpec("core")
        in_specs = (self.pspec,) * (n_params + n_outs)
        out_specs = (self.pspec,) * n_outs
        self.sharded = jax.jit(
            shard_map(_body, mesh=self.mesh, in_specs=in_specs,
                      out_specs=out_specs, check_rep=False),
            donate_argnums=donate, keep_unused=True)
        self.in_names = in_names
        self.out_names = out_names
        self.out_avals = out_avals

    def _concat_in(self, in_maps):
        return [
            np.concatenate([np.asarray(m[name]) for m in in_maps], axis=0)
            for name in self.in_names
        ]

    def _zeros(self):
        return [np.zeros((NC * a.shape[0], *a.shape[1:]), a.dtype)
                for a in self.out_avals]

    def run(self, in_maps):
        out_arrs = self.sharded(*self._concat_in(in_maps), *self._zeros())
        self.jax.block_until_ready(out_arrs)
        return [
            {name: np.asarray(out_arrs[i]).reshape(NC, *self.out_avals[i].shape)[c]
             for i, name in enumerate(self.out_names)}
            for c in range(NC)
        ]

    def bench(self, in_maps, iters=5):
        """Time steady-state execution with device-resident inputs."""
        import time
        jax = self.jax
        from jax.sharding import NamedSharding
        sh = NamedSharding(self.mesh, self.pspec)
        dev_in = [jax.device_put(a, sh) for a in self._concat_in(in_maps)]
        jax.block_until_ready(dev_in)
        times = []
        for _ in range(iters):
            zs = [jax.device_put(z, sh) for z in self._zeros()]
            jax.block_until_ready(zs)
            t0 = time.perf_counter()
            out = self.sharded(*dev_in, *zs)
            jax.block_until_ready(out)
            times.append(time.perf_counter() - t0)
        return min(times), times


def _programs():
    if "run1" not in _CACHE:
        _CACHE["run1"] = _Runner(_build_phase1())
        _CACHE["run2"] = _Runner(_build_phase2())
    return _CACHE["run1"], _CACHE["run2"]


def _chain_fns():
    """Jitted helpers for the single-dispatch chain (cached)."""
    if "resh" in _CACHE:
        return _CACHE["resh"], _CACHE["zeros1"], _CACHE["zeros2"]
    import jax
    import jax.numpy as jnp
    from jax.sharding import NamedSharding

    run1, _ = _programs()
    sh = NamedSharding(run1.mesh, run1.pspec)

    def _reshard(A):
        # A: [NC*GW, S] (core-major attn_t) -> [NC*D, TPC] (core-major at)
        # at[2b+h][g*GW+r, u] = A[(2b+g)*GW + r, h*TPC + u]
        A = A.reshape(B, 2, GW, 2, TPC)      # b, g, r, h, u
        A = A.transpose(0, 3, 1, 2, 4)       # b, h, g, r, u
        return A.reshape(NC * D, TPC)

    _CACHE["resh"] = jax.jit(_reshard, out_shardings=sh)
    _CACHE["zeros1"] = jax.jit(
        lambda: jnp.zeros((NC * GW, S), jnp.float32), out_shardings=sh)
    _CACHE["zeros2"] = jax.jit(
        lambda: jnp.zeros((NC * TPC, D), jnp.float32), out_shardings=sh)
    return _CACHE["resh"], _CACHE["zeros1"], _CACHE["zeros2"]


def _device_inputs(x, wq, bq, wk, bk, wv, bv, wo, bo, ln_g, ln_b):
    """device_put all chain inputs (async); returns (dev1 list, dev2 list)."""
    import jax
    from jax.sharding import NamedSharding

    run1, run2 = _programs()
    sh = NamedSharding(run1.mesh, run1.pspec)
    in1 = _phase1_inputs(x, wq, bq, wk, bk, wv, bv)
    dev1 = [jax.device_put(a, sh) for a in run1._concat_in(in1)]
    x_flat = np.ascontiguousarray(x.reshape(T, D))
    wo_c = np.concatenate([np.asarray(wo, np.float32)] * NC, axis=0)
    bo_c = np.concatenate([np.asarray(bo, np.float32)] * NC, axis=0)
    lng_c = np.concatenate([np.asarray(ln_g, np.float32)] * NC, axis=0)
    lnb_c = np.concatenate([np.asarray(ln_b, np.float32)] * NC, axis=0)
    # order must match run2.in_names sans `at`: wo, xr, bo, lng, lnb
    dev2 = [jax.device_put(a, sh) for a in (wo_c, x_flat, bo_c, lng_c, lnb_c)]
    return dev1, dev2


def _chain(dev1, dev2):
    """Issue the full async chain; returns the (device) output array."""
    run1, run2 = _programs()
    resh, zeros1, zeros2 = _chain_fns()
    (attn,) = run1.sharded(*dev1, zeros1())
    at = resh(attn)
    (out,) = run2.sharded(at, *dev2, zeros2())
    return out


def _masks() -> np.ndarray:
    # sliding-window causal mask: variant r = W[:, 3*KT - r*KT :][:QC]
    # W[j, u] = 0 if j <= u - 3*KT else NEG
    W = np.zeros((KT, 3 * KT + QC), dtype=np.float32)
    j = np.arange(KT)[:, None]
    u = np.arange(3 * KT + QC)[None, :]
    W[j > u - 3 * KT] = NEG
    return W


def _phase1_inputs(x, wq, bq, wk, bk, wv, bv):
    xt = np.ascontiguousarray(x.transpose(0, 2, 1))        # [B, D, S]
    masks = _masks()
    in1 = []
    for c in range(NC):
        b, g = c // 2, c % 2
        sl = slice(g * GW, (g + 1) * GW)
        in1.append({
            "xt": xt[b],
            "wq": np.ascontiguousarray(np.asarray(wq)[:, sl]) * np.float32(0.125),
            "wk": np.ascontiguousarray(np.asarray(wk)[:, sl]),
            "wv": np.ascontiguousarray(np.asarray(wv)[:, sl]),
            "bq": np.ascontiguousarray(np.asarray(bq)[sl]) * np.float32(0.125),
            "bk": np.ascontiguousarray(np.asarray(bk)[sl]),
            "bv": np.ascontiguousarray(np.asarray(bv)[sl]),
            "masks": masks,
        })
    return in1


def kernel(x, wq, bq, wk, bk, wv, bv, wo, bo, ln_g, ln_b, _profile=None):
    import time as _time
    x = np.asarray(x, np.float32)

    t0 = _time.perf_counter()
    dev1, dev2 = _device_inputs(x, wq, bq, wk, bk, wv, bv, wo, bo, ln_g, ln_b)
    t1 = _time.perf_counter()
    out_dev = _chain(dev1, dev2)
    out = np.asarray(out_dev)          # single blocking fetch
    t2 = _time.perf_counter()
    if _profile is not None:
        _profile["t_upload"] = t1 - t0
        _profile["t_chain"] = t2 - t1
        _profile["dev1"], _profile["dev2"] = dev1, dev2
    return out.reshape(B, S, D)

